# revision 6
# baseline (speedup 1.0000x reference)
"""BLSTM generator kernel v2 — packed scans.

Key changes vs baseline:
- Scan recurrent matmuls packed into 4 PE col-groups via tile_position:
  chain (cell c, unit-half u) -> col-group 2u+c at psum partitions 32*(2u+c).
  Gate columns reordered per unit-half: [i(256)|f(256)|o(256)|g(256)] so each
  (c,u) chain owns a contiguous 1024-col block (2 psum banks).
- Elementwise stacked across cells in the partition dim: Act/DVE ops cover
  both cells per op ([48,*] / [112,*]), so engine time is ~cell-count
  independent.
- h transposed via PE transpose-mode into psum, one DVE copy into a per-cell
  accumulator tile that doubles as (a) the stationary hT source for the next
  step's Wh matmuls and (b) the DMA export buffer (8-step chunks).
"""
import sys, os
sys.path.insert(0, '/opt/trn_rl_repo')
import numpy as np
import ml_dtypes

import concourse.bass as bass
import concourse.bacc as bacc
import concourse.mybir as mybir
import concourse.tile as tile
from concourse import bass_utils

BF16 = ml_dtypes.bfloat16
V, E, H, B, T = 512, 256, 512, 128, 512
NCORES = 8
BL = B // NCORES          # 16
H4 = 4 * H                # 2048
CH = 8                    # export chunk (steps)

AF = mybir.ActivationFunctionType
DT = mybir.dt
ADD = mybir.AluOpType.add
MULT = mybir.AluOpType.mult

_CACHE = {}


def _gate_perm():
    # device column j = u*1024 + blk*256 + r  (blk: 0=i,1=f,2=o,3=g; u half)
    # reference gate order [i, f, g, o] (each 512 wide)
    srcbase = {0: 0, 1: 512, 2: 3 * 512, 3: 2 * 512}
    perm = np.empty(H4, np.int64)
    for u in range(2):
        for blk in range(4):
            d0 = u * 1024 + blk * 256
            perm[d0:d0 + 256] = srcbase[blk] + u * 256 + np.arange(256)
    return perm


# ---------------------------------------------------------------------------
# device program
# ---------------------------------------------------------------------------

def _bigmm(nc, tc, name, lhsT_dram, wx_sb, k_tiles, bias_sb, out_writer,
           n_mtiles, extra_cells=None):
    """Gx = lhsT.T @ Wx + bias -> bf16 out.  lhsT_dram: [k_tiles*128, n_mtiles*128].
    wx_sb: sbuf [128, k_tiles*H4].  out_writer(m, n, tile)."""
    cells = [(wx_sb, bias_sb, out_writer)] + (extra_cells or [])
    with tc.tile_pool(name=f"{name}_lhs", bufs=3) as lp, \
         tc.tile_pool(name=f"{name}_ps", bufs=4, space="PSUM") as pp, \
         tc.tile_pool(name=f"{name}_ev", bufs=4) as ep:
        evn = 0
        for m in range(n_mtiles):
            lts = []
            for k in range(k_tiles):
                lt = lp.tile([128, 128], DT.bfloat16, tag=f"lhs{k}", name=f"lhs_{k}")
                nc.sync.dma_start(lt[:], lhsT_dram[k * 128:(k + 1) * 128,
                                                   m * 128:(m + 1) * 128])
                lts.append(lt)
            for (wsb, bsb, wr) in cells:
                for n in range(4):
                    ps = pp.tile([128, 512], DT.float32, tag="ps")
                    for k in range(k_tiles):
                        nc.tensor.matmul(ps[:], lts[k][:],
                                         wsb[:, k * H4 + n * 512: k * H4 + (n + 1) * 512],
                                         start=(k == 0), stop=(k == k_tiles - 1))
                    ev = ep.tile([128, 512], DT.bfloat16, tag="ev")
                    nc.vector.tensor_tensor(ev[:], ps[:], bsb[:, n * 512:(n + 1) * 512], op=ADD)
                    evn += 1
                    wr(m, n, ev)


def _gx_writer(nc, gx_dram):
    def wr(m, n, ev):
        nc.sync.dma_start(gx_dram[m * 128:(m + 1) * 128, n * 512:(n + 1) * 512],
                          ev[:])
    return wr


class ScanResult:
    def __init__(self):
        self.acc = None       # list per cell: final acc tile
        self.pos = None       # list per cell: column pos (0..7) of final step
        self.c = None         # c tile [128, 256] f32; blocks at P(c,u)


def _P(c, u):
    return 32 * (2 * u + c)


def _scan(nc, tc, name, TS, ncells, gx_drams, revs, wh_sbs, ident_sb,
          exports, init=None):
    """Packed scan.  ncells in {1,2}.  gx_drams[c]: dram [TS*BL, H4] bf16.
    revs[c]: reversed scan.  wh_sbs[c]: sbuf [128, 4*H4] bf16 (cols permuted).
    exports[c]: None or dram AP [512 rows offset applied by caller, TS*16].
    init: None or (dec_hT_sb [128, 64] bf16, dec_c_sb [16, 512] f32)."""
    res = ScanResult()
    rmax = 64 + 48 if ncells == 2 else 64 + 16   # elementwise partition extent
    with tc.tile_pool(name=f"{name}_gx", bufs=4) as gxp, \
         tc.tile_pool(name=f"{name}_ps", bufs=1, space="PSUM") as psp, \
         tc.tile_pool(name=f"{name}_pt", bufs=2, space="PSUM") as ptp, \
         tc.tile_pool(name=f"{name}_act", bufs=2) as ap, \
         tc.tile_pool(name=f"{name}_st", bufs=2) as sp, \
         tc.tile_pool(name=f"{name}_acc", bufs=2) as accp:
        ps = psp.tile([128, H4], DT.float32, tag="ps")
        # zero the never-written partition rows once so wide Act reads are clean
        nc.vector.memset(ps[:], 0.0)

        prev_acc = [None] * ncells
        prev_pos = [None] * ncells
        c_prev = None
        if init is not None:
            dec_hT_sb, dec_c_sb = init
            for c in range(ncells):
                acc0 = accp.tile([128, 512], DT.bfloat16, tag=f"acc{c}",
                                 name=f"acc_init_{c}")
                src = dec_hT_sb[:].rearrange("p (k s) -> p k s", k=4)
                nc.vector.tensor_copy(
                    acc0[:].rearrange("p (k s) -> p k s", k=4)[:, :, 112:128],
                    src)
                prev_acc[c] = acc0
                prev_pos[c] = 7
            c_prev = sp.tile([128, 256], DT.float32, tag="c", name="c_init")
            nc.vector.memset(c_prev[:], 0.0)
            for c in range(ncells):
                for u in range(2):
                    p0 = _P(c, u)
                    nc.vector.tensor_copy(c_prev[p0:p0 + 16, :],
                                          dec_c_sb[0:16, u * 256:(u + 1) * 256])

        acc = [None] * ncells
        for s in range(TS):
            ci = s % CH
            if ci == 0:
                for c in range(ncells):
                    acc[c] = accp.tile([128, 512], DT.bfloat16, tag=f"acc{c}",
                                       name=f"acc_{c}")
            gxt = [None] * ncells
            for c in range(ncells):
                t = (TS - 1 - s) if revs[c] else s
                gxt[c] = gxp.tile([16, H4], DT.bfloat16, tag=f"gx{c}",
                                  name=f"gx_{c}")
                nc.sync.dma_start(gxt[c][:], gx_drams[c][t * BL:(t + 1) * BL, :])
            # matmul chains: (c, u) -> col-group 2u+c.  Emission is
            # interleaved across groups so consecutive PE instructions hit
            # different col-groups and overlap in the array (per-group order
            # is preserved, which is what PSUM accumulation requires).
            chains = [(c, u) for c in range(ncells) for u in range(2)]
            for nsub in range(2):
                for step_i in range(5):
                    for (c, u) in chains:
                        P = _P(c, u)
                        cb = u * 1024 + nsub * 512
                        out_ap = ps[P:P + 16, cb:cb + 512]
                        if step_i == 0:
                            nc.tensor.matmul(out_ap, ident_sb[0:16, 0:16],
                                             gxt[c][0:16, cb:cb + 512],
                                             start=True,
                                             stop=(prev_acc[c] is None),
                                             tile_position=(0, P))
                        elif prev_acc[c] is not None:
                            k = step_i - 1
                            pp16 = prev_pos[c] * 16
                            nc.tensor.matmul(
                                out_ap,
                                prev_acc[c][:, k * 128 + pp16: k * 128 + pp16 + 16],
                                wh_sbs[c][:, k * H4 + cb: k * H4 + cb + 512],
                                start=False, stop=(k == 3),
                                tile_position=(0, P))
            # activations: per u-half, both cells stacked in partitions
            sig = ap.tile([128, 768], DT.float32, tag="sig", name="sig")
            tng = ap.tile([128, 256], DT.float32, tag="tng", name="tng")
            span = 48 if ncells == 2 else 16
            for u in range(2):
                p0 = 64 * u
                # u0 spans [0:64] so rows 48:64 (never matmul-written, memset
                # to 0) are initialized for the stacked [0:rmax] elementwise
                aspan = 64 if u == 0 else span
                nc.scalar.activation(sig[p0:p0 + aspan, :],
                                     ps[p0:p0 + aspan, u * 1024:u * 1024 + 768],
                                     AF.Sigmoid)
                nc.scalar.activation(tng[p0:p0 + aspan, :],
                                     ps[p0:p0 + aspan, u * 1024 + 768:(u + 1) * 1024],
                                     AF.Tanh)
            # elementwise (rows 0:rmax cover all cell/half blocks; junk lanes ok)
            R = slice(0, rmax)
            t1 = sp.tile([128, 256], DT.float32, tag="t1", name="t1")
            nc.vector.tensor_tensor(t1[R, :], sig[R, 0:256], tng[R, :], op=MULT)
            c_new = sp.tile([128, 256], DT.float32, tag="c", name="c")
            if c_prev is not None:
                t2 = sp.tile([128, 256], DT.float32, tag="t2", name="t2")
                nc.gpsimd.tensor_tensor(t2[R, :], sig[R, 256:512], c_prev[R, :],
                                        op=MULT)
                nc.vector.tensor_tensor(c_new[R, :], t1[R, :], t2[R, :], op=ADD)
            else:
                nc.vector.tensor_copy(c_new[R, :], t1[R, :])
            tch = sp.tile([128, 256], DT.float32, tag="tch", name="tch")
            nc.scalar.activation(tch[R, :], c_new[R, :], AF.Tanh)
            h_t = sp.tile([128, 256], DT.bfloat16, tag="h", name="h")
            nc.vector.tensor_tensor(h_t[R, :], sig[R, 512:768], tch[R, :], op=MULT)
            # transpose h: two full-height base-0 transposes (HW rejects
            # mixing transpose input partition bases within a program)
            psT = ptp.tile([128, 256], DT.bfloat16, tag="psT", name="psT")
            for half in range(2):
                nc.tensor.transpose(psT[:, half * 128:half * 128 + rmax],
                                    h_t[0:rmax, half * 128:(half + 1) * 128],
                                    ident_sb[0:rmax, 0:rmax])
            # acc copies (stationary + export source), cast to bf16
            # psT col layout: half*128 + u*64 + 32*c (+0:16)
            # acc col layout: k*128 + pos*16 with chunk k = 2*u + half
            psT4 = psT[:].rearrange("p (hf u2 s) -> p hf u2 s", hf=2, u2=2)
            for c in range(ncells):
                pos = (CH - 1 - ci) if revs[c] else ci
                acc4 = acc[c][:].rearrange("p (u2 hf2 s) -> p u2 hf2 s",
                                           u2=2, hf2=2)
                for half in range(2):
                    nc.vector.tensor_copy(
                        acc4[:, :, half, pos * 16:(pos + 1) * 16],
                        psT4[:, half, :, 32 * c:32 * c + 16])
                prev_acc[c] = acc[c]
                prev_pos[c] = pos
            c_prev = c_new
            # export finished chunks
            if ci == CH - 1:
                for c in range(ncells):
                    if exports[c] is not None:
                        t0 = (TS - s - 1) if revs[c] else (s - CH + 1)
                        for k in range(4):
                            nc.sync.dma_start(
                                exports[c][k * 128:(k + 1) * 128,
                                           t0 * 16:(t0 + CH) * 16],
                                acc[c][:, k * 128:(k + 1) * 128])
        res.acc = prev_acc
        res.pos = prev_pos
        res.c = c_prev
    return res


def _build(TS):
    nc = bacc.Bacc("TRN2", target_bir_lowering=False, debug=False,
                   enable_asserts=False, num_devices=NCORES)
    f32, bf16 = DT.float32, DT.bfloat16

    def din(name, shape, dt=bf16):
        return nc.dram_tensor(name, shape, dt, kind="ExternalInput").ap()

    eT = din("eT", [E, TS * BL])
    wx = {c: din(f"wx_{c}", [insz, H4]) for c, insz in
          [("f0", E), ("b0", E), ("d0", E), ("f1", 2 * H), ("b1", 2 * H), ("d1", H)]}
    wh = {c: din(f"wh_{c}", [H, H4]) for c in ["f0", "b0", "f1", "b1", "d0", "d1"]}
    bias = {c: din(f"bias_{c}", [128, H4], f32) for c in
            ["f0", "b0", "f1", "b1", "d0", "d1"]}
    hproj_w = din("hproj_w", [2 * H, H])
    cproj_w = din("cproj_w", [2 * H, H])
    hproj_b = din("hproj_b", [128, H], f32)
    cproj_b = din("cproj_b", [128, H], f32)
    fc_w = din("fc_w", [H, V])
    fc_b = din("fc_b", [128, V], f32)
    ident = din("ident", [128, 128])
    logits = nc.dram_tensor("logits", [BL, TS, V], bf16, kind="ExternalOutput").ap()

    with tile.TileContext(nc) as tc:
        with tc.tile_pool(name="dram", bufs=1, space="DRAM") as dp, \
             tc.tile_pool(name="const", bufs=1) as cp:
            gx = {c: dp.tile([TS * BL, H4], bf16, tag=f"gx_{c}", name=f"gx_{c}")
                  for c in ["f0", "b0", "f1", "b1", "d0", "d1"]}
            liT = dp.tile([2 * H, TS * BL], bf16, tag="liT")
            h0T = dp.tile([H, TS * BL], bf16, tag="h0T")
            h1T = dp.tile([H, TS * BL], bf16, tag="h1T")

            ident_sb = cp.tile([128, 128], bf16)
            nc.sync.dma_start(ident_sb[:], ident)

            n_mt = TS * BL // 128

            # ---- phase 1: Gx for f0, b0, d0 from eT ----
            with tc.tile_pool(name="p1w", bufs=1) as wp:
                wx_sb, bias_sb = {}, {}
                for c in ["f0", "b0", "d0"]:
                    wx_sb[c] = wp.tile([128, 2 * H4], bf16, tag=f"wx_{c}", name=f"wxsb_{c}")
                    for k in range(2):
                        nc.sync.dma_start(wx_sb[c][:, k * H4:(k + 1) * H4],
                                          wx[c][k * 128:(k + 1) * 128, :])
                    bias_sb[c] = wp.tile([128, H4], f32, tag=f"bs_{c}", name=f"bsb_{c}")
                    nc.sync.dma_start(bias_sb[c][:], bias[c])
                _bigmm(nc, tc, "p1", eT, wx_sb["f0"], 2, bias_sb["f0"],
                       _gx_writer(nc, gx["f0"]), n_mt,
                       extra_cells=[(wx_sb["b0"], bias_sb["b0"], _gx_writer(nc, gx["b0"])),
                                    (wx_sb["d0"], bias_sb["d0"], _gx_writer(nc, gx["d0"]))])

            # ---- phase 2: L0 scan ----
            with tc.tile_pool(name="p2w", bufs=1) as wp:
                wh_sb = {}
                for c in ["f0", "b0"]:
                    wh_sb[c] = wp.tile([128, 4 * H4], bf16, tag=f"wh_{c}", name=f"whsb_{c}")
                    for k in range(4):
                        nc.sync.dma_start(wh_sb[c][:, k * H4:(k + 1) * H4],
                                          wh[c][k * 128:(k + 1) * 128, :])
                _scan(nc, tc, "l0", TS, 2, [gx["f0"], gx["b0"]], [False, True],
                      [wh_sb["f0"], wh_sb["b0"]], ident_sb,
                      [liT[0:H, :], liT[H:2 * H, :]])

            # ---- phase 3: Gx for f1, b1 from liT ----
            with tc.tile_pool(name="p3w", bufs=1) as wp:
                wx_sb, bias_sb = {}, {}
                for c in ["f1", "b1"]:
                    wx_sb[c] = wp.tile([128, 8 * H4], bf16, tag=f"wx_{c}", name=f"wxsb_{c}")
                    for k in range(8):
                        nc.sync.dma_start(wx_sb[c][:, k * H4:(k + 1) * H4],
                                          wx[c][k * 128:(k + 1) * 128, :])
                    bias_sb[c] = wp.tile([128, H4], f32, tag=f"bs_{c}", name=f"bsb_{c}")
                    nc.sync.dma_start(bias_sb[c][:], bias[c])
                _bigmm(nc, tc, "p3", liT, wx_sb["f1"], 8, bias_sb["f1"],
                       _gx_writer(nc, gx["f1"]), n_mt,
                       extra_cells=[(wx_sb["b1"], bias_sb["b1"], _gx_writer(nc, gx["b1"]))])

            # ---- phase 4: L1 scan ----
            with tc.tile_pool(name="p4w", bufs=1) as wp:
                wh_sb = {}
                for c in ["f1", "b1"]:
                    wh_sb[c] = wp.tile([128, 4 * H4], bf16, tag=f"wh_{c}", name=f"whsb_{c}")
                    for k in range(4):
                        nc.sync.dma_start(wh_sb[c][:, k * H4:(k + 1) * H4],
                                          wh[c][k * 128:(k + 1) * 128, :])
                enc = _scan(nc, tc, "l1", TS, 2, [gx["f1"], gx["b1"]], [False, True],
                            [wh_sb["f1"], wh_sb["b1"]], ident_sb, [None, None])

            # ---- phase 5: bridge ----
            dec_hT = cp.tile([128, 64], bf16, tag="dec_hT")
            dec_c = cp.tile([32, 512], f32, tag="dec_c")
            with tc.tile_pool(name="br", bufs=1) as brp, \
                 tc.tile_pool(name="br_ps", bufs=2, space="PSUM") as brps:
                pw_sb = brp.tile([128, 8 * H], bf16, tag="pw")
                cw_sb = brp.tile([128, 8 * H], bf16, tag="cw")
                # row chunk (c*4 + k) = rows (c*512 + k*128)
                for c in range(2):
                    for k in range(4):
                        r0 = c * 512 + k * 128
                        nc.sync.dma_start(pw_sb[:, (c * 4 + k) * H:(c * 4 + k + 1) * H],
                                          hproj_w[r0:r0 + 128, :])
                        nc.sync.dma_start(cw_sb[:, (c * 4 + k) * H:(c * 4 + k + 1) * H],
                                          cproj_w[r0:r0 + 128, :])
                pb_sb = brp.tile([128, H], f32, tag="pb")
                cb_sb = brp.tile([128, H], f32, tag="cb")
                nc.sync.dma_start(pb_sb[:], hproj_b)
                nc.sync.dma_start(cb_sb[:], cproj_b)
                # c -> bf16 -> transposed chunks in sbuf
                c_bf = brp.tile([128, 256], bf16, tag="cbf")
                nc.vector.tensor_copy(c_bf[0:112, :], enc.c[0:112, :])
                psT2 = brps.tile([128, 256], bf16, tag="psT2")
                for half in range(2):
                    nc.tensor.transpose(psT2[:, half * 128:half * 128 + 112],
                                        c_bf[0:112, half * 128:(half + 1) * 128],
                                        ident_sb[0:112, 0:112])
                cT_sb = brp.tile([128, 256], bf16, tag="cT")
                nc.vector.tensor_copy(cT_sb[:, 0:112], psT2[:, 0:112])
                nc.vector.tensor_copy(cT_sb[:, 128:240], psT2[:, 128:240])
                # projections
                ps_h = brps.tile([16, H], f32, tag="psh")
                ps_c = brps.tile([16, H], f32, tag="psc")
                for c in range(2):
                    p16 = enc.pos[c] * 16
                    for k in range(4):
                        nc.tensor.matmul(ps_h[:], enc.acc[c][:, k * 128 + p16:k * 128 + p16 + 16],
                                         pw_sb[:, (c * 4 + k) * H:(c * 4 + k + 1) * H],
                                         start=(c == 0 and k == 0), stop=(c == 1 and k == 3))
                for c in range(2):
                    for k in range(4):
                        cc = (k % 2) * 128 + (k // 2) * 64 + 32 * c
                        nc.tensor.matmul(ps_c[:], cT_sb[:, cc:cc + 16],
                                         cw_sb[:, (c * 4 + k) * H:(c * 4 + k + 1) * H],
                                         start=(c == 0 and k == 0), stop=(c == 1 and k == 3))
                tmp = brp.tile([32, 512], f32, tag="tmp")
                nc.vector.tensor_tensor(tmp[0:16, :], ps_h[:], pb_sb[0:16, :], op=ADD)
                dec_h = brp.tile([32, 512], bf16, tag="dec_h")
                nc.scalar.activation(dec_h[0:16, :], tmp[0:16, :], AF.Tanh)
                tmp2 = brp.tile([32, 512], f32, tag="tmp2")
                nc.vector.tensor_tensor(tmp2[0:16, :], ps_c[:], cb_sb[0:16, :], op=ADD)
                nc.scalar.activation(dec_c[0:16, :], tmp2[0:16, :], AF.Tanh)
                psT3 = brps.tile([128, 64], bf16, tag="psT3")
                for k in range(4):
                    nc.tensor.transpose(psT3[:, k * 16:k * 16 + 16],
                                        dec_h[0:16, k * 128:(k + 1) * 128],
                                        ident_sb[0:16, 0:16])
                nc.vector.tensor_copy(dec_hT[:], psT3[:])

            # ---- phase 6: d0 scan ----
            with tc.tile_pool(name="p6w", bufs=1) as wp:
                wh_sb = wp.tile([128, 4 * H4], bf16, tag="wh_d0")
                for k in range(4):
                    nc.sync.dma_start(wh_sb[:, k * H4:(k + 1) * H4],
                                      wh["d0"][k * 128:(k + 1) * 128, :])
                _scan(nc, tc, "d0", TS, 1, [gx["d0"]], [False],
                      [wh_sb], ident_sb, [h0T],
                      init=(dec_hT, dec_c[0:16, :]))

            # ---- phase 7: Gx for d1 from h0T ----
            with tc.tile_pool(name="p7w", bufs=1) as wp:
                wx_sb = wp.tile([128, 4 * H4], bf16, tag="wx_d1")
                for k in range(4):
                    nc.sync.dma_start(wx_sb[:, k * H4:(k + 1) * H4],
                                      wx["d1"][k * 128:(k + 1) * 128, :])
                bias_sb = wp.tile([128, H4], f32, tag="bs_d1")
                nc.sync.dma_start(bias_sb[:], bias["d1"])
                _bigmm(nc, tc, "p7", h0T, wx_sb, 4, bias_sb,
                       _gx_writer(nc, gx["d1"]), n_mt)

            # ---- phase 8: d1 scan ----
            with tc.tile_pool(name="p8w", bufs=1) as wp:
                wh_sb = wp.tile([128, 4 * H4], bf16, tag="wh_d1")
                for k in range(4):
                    nc.sync.dma_start(wh_sb[:, k * H4:(k + 1) * H4],
                                      wh["d1"][k * 128:(k + 1) * 128, :])
                _scan(nc, tc, "d1", TS, 1, [gx["d1"]], [False],
                      [wh_sb], ident_sb, [h1T],
                      init=(dec_hT, dec_c[0:16, :]))

            # ---- phase 9: FC ----
            with tc.tile_pool(name="p9w", bufs=1) as wp:
                fc_sb = wp.tile([128, 4 * V], bf16, tag="fc_w")
                for k in range(4):
                    nc.sync.dma_start(fc_sb[:, k * V:(k + 1) * V],
                                      fc_w[k * 128:(k + 1) * 128, :])
                fcb_sb = wp.tile([128, V], f32, tag="fc_b")
                nc.sync.dma_start(fcb_sb[:], fc_b)
                with tc.tile_pool(name="fc_l", bufs=3) as lp, \
                     tc.tile_pool(name="fc_ps", bufs=4, space="PSUM") as pp, \
                     tc.tile_pool(name="fc_ev", bufs=4) as ep:
                    for m in range(n_mt):
                        lts = []
                        for k in range(4):
                            lt = lp.tile([128, 128], bf16, tag=f"l{k}", name=f"fcl_{k}")
                            nc.sync.dma_start(lt[:], h1T[k * 128:(k + 1) * 128,
                                                         m * 128:(m + 1) * 128])
                            lts.append(lt)
                        ps = pp.tile([128, V], f32, tag="ps")
                        for k in range(4):
                            nc.tensor.matmul(ps[:], lts[k][:], fc_sb[:, k * V:(k + 1) * V],
                                             start=(k == 0), stop=(k == 3))
                        ev = ep.tile([128, V], bf16, tag="ev")
                        nc.vector.tensor_tensor(ev[:], ps[:], fcb_sb[:], op=ADD)
                        dst = logits[0:BL, m * 8:(m + 1) * 8, :].rearrange("b t v -> t b v")
                        nc.sync.dma_start(dst, ev[:])

    nc.compile()
    return nc


# ---------------------------------------------------------------------------
# host wrapper
# ---------------------------------------------------------------------------

def _prep_inputs(inputs, T_steps):
    perm = _gate_perm()
    x = np.asarray(inputs["x"])
    emb = np.asarray(inputs["emb"], np.float32)
    e = emb[x][:, :T_steps]                     # [B, T, E] fp32

    def wp(wname):
        return np.ascontiguousarray(
            np.asarray(inputs[wname], np.float32)[:, perm]).astype(BF16)

    def bp(bname):
        b = np.asarray(inputs[bname], np.float32)[perm]
        return np.ascontiguousarray(np.broadcast_to(b, (128, H4))).astype(np.float32)

    cells = {"f0": ("enc_f", "0"), "b0": ("enc_b", "0"), "f1": ("enc_f", "1"),
             "b1": ("enc_b", "1"), "d0": ("dec", "0"), "d1": ("dec", "1")}
    common = {}
    for c, (pre, li) in cells.items():
        common[f"wx_{c}"] = wp(f"{pre}_Wx{li}")
        common[f"wh_{c}"] = wp(f"{pre}_Wh{li}")
        common[f"bias_{c}"] = bp(f"{pre}_b{li}")
    common["hproj_w"] = np.asarray(inputs["hproj_W"], np.float32).astype(BF16)
    common["cproj_w"] = np.asarray(inputs["cproj_W"], np.float32).astype(BF16)
    common["hproj_b"] = np.ascontiguousarray(np.broadcast_to(
        np.asarray(inputs["hproj_b"], np.float32), (128, H))).astype(np.float32)
    common["cproj_b"] = np.ascontiguousarray(np.broadcast_to(
        np.asarray(inputs["cproj_b"], np.float32), (128, H))).astype(np.float32)
    common["fc_w"] = np.asarray(inputs["fc_W"], np.float32).astype(BF16)
    common["fc_b"] = np.ascontiguousarray(np.broadcast_to(
        np.asarray(inputs["fc_b"], np.float32), (128, V))).astype(np.float32)
    common["ident"] = np.eye(128, dtype=np.float32).astype(BF16)

    in_maps = []
    for c in range(NCORES):
        m = dict(common)
        ec = e[c * BL:(c + 1) * BL]                    # [BL, T, E]
        m["eT"] = np.ascontiguousarray(
            ec.transpose(2, 1, 0).reshape(E, T_steps * BL)).astype(BF16)
        in_maps.append(m)
    return in_maps


# -- direct PJRT runner: device-created output buffers (no zero upload), ----
# -- and a device-staged timing path ----------------------------------------

def _make_exec(nc, n_cores=NCORES):
    import jax
    from jax.experimental.shard_map import shard_map
    from jax.sharding import Mesh, PartitionSpec, NamedSharding
    import jax.numpy as jnp
    from concourse.bass2jax import (install_neuronx_cc_hook,
                                    partition_id_tensor, _bass_exec_p)
    install_neuronx_cc_hook()
    partition_name = (nc.partition_id_tensor.name
                      if nc.partition_id_tensor else None)
    in_names, out_names, out_avals = [], [], []
    for alloc in nc.m.functions[0].allocations:
        if not isinstance(alloc, mybir.MemoryLocationSet):
            continue
        name = alloc.memorylocations[0].name
        if alloc.kind == "ExternalInput":
            if name != partition_name:
                in_names.append(name)
        elif alloc.kind == "ExternalOutput":
            out_names.append(name)
            out_avals.append(jax.core.ShapedArray(
                tuple(alloc.tensor_shape), mybir.dt.np(alloc.dtype)))
    n_params = len(in_names)
    n_outs = len(out_names)
    bind_in_names = list(in_names) + list(out_names)
    if partition_name is not None:
        bind_in_names.append(partition_name)

    def _body(*args):
        operands = list(args)
        if partition_name is not None:
            operands.append(partition_id_tensor())
        outs = _bass_exec_p.bind(
            *operands, out_avals=tuple(out_avals),
            in_names=tuple(bind_in_names), out_names=tuple(out_names),
            lowering_input_output_aliases=(), sim_require_finite=True,
            sim_require_nnan=True, nc=nc)
        return tuple(outs)

    devices = jax.devices()[:n_cores]
    mesh = Mesh(np.asarray(devices), ("core",))
    donate = tuple(range(n_params, n_params + n_outs))
    sharded = jax.jit(
        shard_map(_body, mesh=mesh,
                  in_specs=(PartitionSpec("core"),) * (n_params + n_outs),
                  out_specs=(PartitionSpec("core"),) * n_outs,
                  check_rep=False),
        donate_argnums=donate, keep_unused=True)
    sh = NamedSharding(mesh, PartitionSpec("core"))
    mk_zeros = jax.jit(
        lambda: tuple(jnp.zeros((n_cores * a.shape[0], *a.shape[1:]), a.dtype)
                      for a in out_avals),
        out_shardings=(sh,) * n_outs if n_outs > 1 else sh)
    return dict(in_names=in_names, out_names=out_names, out_avals=out_avals,
                sharded=sharded, mk_zeros=mk_zeros, sh=sh, n_cores=n_cores)


_EXEC_CACHE = {}


def _get_exec(T_steps):
    if T_steps not in _CACHE:
        _CACHE[T_steps] = _build(T_steps)
    if T_steps not in _EXEC_CACHE:
        _EXEC_CACHE[T_steps] = _make_exec(_CACHE[T_steps])
    return _EXEC_CACHE[T_steps]


def _concat_inputs(ex, in_maps):
    n = len(in_maps)
    return [np.concatenate([np.asarray(in_maps[c][name]) for c in range(n)],
                           axis=0) for name in ex["in_names"]]


def _exec_outs_to_logits(ex, outs, T_steps):
    import jax
    li = ex["out_names"].index("logits")
    g = np.asarray(outs[li], dtype=np.float32)    # [8*BL, T, V]
    return g.reshape(B, T_steps, V)


def run(inputs, T_steps=T, trace=False):
    ex = _get_exec(T_steps)
    in_maps = _prep_inputs(inputs, T_steps)
    concat = _concat_inputs(ex, in_maps)
    zs = ex["mk_zeros"]()
    if not isinstance(zs, tuple):
        zs = (zs,)
    outs = ex["sharded"](*concat, *zs)
    return _exec_outs_to_logits(ex, outs, T_steps), None


def timed_device_run(inputs, T_steps=T, iters=3):
    """Stage inputs on-device once, then measure steady-state per-run device
    execution time: N async dispatches are queued back-to-back and the
    marginal time per extra run (slope) removes the fixed dispatch RTT.
    Returns (seconds_per_run, logits)."""
    import jax, time as _time
    ex = _get_exec(T_steps)
    in_maps = _prep_inputs(inputs, T_steps)
    concat = _concat_inputs(ex, in_maps)
    dev_in = [jax.device_put(a, ex["sh"]) for a in concat]
    jax.block_until_ready(dev_in)

    def _zs():
        zs = ex["mk_zeros"]()
        return zs if isinstance(zs, tuple) else (zs,)

    # warmup
    outs = ex["sharded"](*dev_in, *_zs())
    jax.block_until_ready(outs)

    def run_n(n):
        zss = [_zs() for _ in range(n)]
        for z in zss:
            jax.block_until_ready(z)
        t0 = _time.perf_counter()
        outs = None
        for z in zss:
            outs = ex["sharded"](*dev_in, *z)
        jax.block_until_ready(outs)
        return _time.perf_counter() - t0, outs

    n_lo, n_hi = 1, 1 + max(2, iters)
    t_lo, _ = run_n(n_lo)
    t_hi, outs = run_n(n_hi)
    per_run = max((t_hi - t_lo) / (n_hi - n_lo), 1e-9)
    return per_run, _exec_outs_to_logits(ex, outs, T_steps)


def kernel(**inputs) -> np.ndarray:
    out, _ = run(inputs, T)
    return out

